# revision 5
# baseline (speedup 1.0000x reference)
"""Trainium2 Bass kernel for nn_InvNUConv2d: label-grouped 1x1 conv.

  y[b, :, h, w] = weight[labels[b, h, w]] @ x[b, :, h, w] + bias[labels[b, h, w]]

Shapes (hardcoded): x [4, 16, 256, 256] f32, labels [4, 256, 256] i32,
weight [25, 16, 16] f32, bias [25] f32 (zeros).

Sharding: 8 cores, each takes half an image in H: core k -> (b = k//2,
h in [128*(k%2), 128*(k%2)+128)) = 32768 pixels x 16 channels.

Device algorithm per core (channel-major end to end, fp16 data path):
  - x arrives as [128, 4096] fp16: partition (g*16+ch) holds channel ch of
    pixel group g (8 groups of 4096 pixels); pure layout reshape on host.
  - pixels are processed in 4 chunks of 1024 per group; each chunk is
    label-sorted into a rank-capped slot space (labels renumbered by
    per-chunk frequency rank; slot range r holds the rank-r label and is
    sized by a static per-rank cap profile) with gpsimd.local_scatter
    (per-partition indices, zero-fill, bulk streaming via Q7 local RAM).
  - one fp16 matmul per (rank, chunk); the 128x128 block-diag stationary
    holds, per 16-channel group block, the weight of that group's rank-r
    label (host-gathered), so ranks can differ across groups and chunks.
  - PSUM -> SBUF fp16 copy (vector/scalar alternating), then a second
    local_scatter restores raster order (pad slots carry idx=-1, skipped).
  - direct DMA out; host undoes the layout reshape and casts to f32.

Host does sharding/layout + index construction from labels; all data
movement and FLOPs run on the NeuronCores.
"""
import numpy as np

import jax
import concourse.bacc as bacc
import concourse.bass as bass  # noqa: F401
import concourse.mybir as mybir
import concourse.tile as tile
from concourse import bass2jax
from jax.sharding import Mesh, PartitionSpec
from jax.experimental.shard_map import shard_map

B, C, H, W, L = 4, 16, 256, 256, 25
N_CORES = 8
NPIX = B * H * W // N_CORES  # 32768 pixels per core
NG = 8                       # partition groups (16 channels each)
GP = NPIX // NG              # 4096 pixels per group
CHK = 1024                   # pixels per chunk (per group)
NCHK = GP // CHK             # 4 chunks

# Per-rank slot caps: rank r holds the r-th most frequent label of each
# (group, chunk); cap = max observed count at that rank (even-rounded).
DEFAULT_PROFILE = (64, 60, 56, 54, 52, 50, 48, 48, 46, 46, 46, 44, 44, 44,
                   42, 42, 42, 40, 40, 40, 40, 38, 38, 38, 38)

F32 = mybir.dt.float32
F16 = mybir.dt.float16
I16 = mybir.dt.int16

PSUM_BANK = 512  # f32 columns per PSUM bank


def _layout(profile):
    """Rank range offsets avoiding PSUM bank straddles; returns (offs, M)."""
    offs = []
    s = 0
    for cap in profile:
        if cap > 0 and (s % PSUM_BANK) + cap > PSUM_BANK:
            s = ((s // PSUM_BANK) + 1) * PSUM_BANK
        offs.append(s)
        s += cap
    m = ((s + 1) // 2) * 2
    return tuple(offs), m


def _build_module(iters=1, profile=DEFAULT_PROFILE):
    offs, M = _layout(profile)
    assert M <= 2046, f"slot space {M} exceeds local_scatter window"
    nc = bacc.Bacc("TRN2", target_bir_lowering=False, debug=False, num_devices=N_CORES)
    x8 = nc.dram_tensor("x8", [128, GP], F16, kind="ExternalInput").ap()
    gidx = nc.dram_tensor("gidx", [128, GP], I16, kind="ExternalInput").ap()
    yinv = nc.dram_tensor("yinv", [128, NCHK * M], I16, kind="ExternalInput").ap()
    wbd = nc.dram_tensor("wbd", [NCHK * L, 128, 128], F16, kind="ExternalInput").ap()
    y8 = nc.dram_tensor("y8", [128, GP], F16, kind="ExternalOutput").ap()

    with tile.TileContext(nc) as tc:
        with (
            tc.tile_pool(name="persist", bufs=1) as pp,
            tc.tile_pool(name="xs_p", bufs=3) as xsp,
            tc.tile_pool(name="ys_p", bufs=5) as ysp,
            tc.tile_pool(name="psum_mm", bufs=2, space="PSUM") as pmp,
        ):
            x_t = pp.tile([128, GP], F16)
            gidx_t = pp.tile([128, GP], I16)
            yinv_t = pp.tile([128, NCHK * M], I16)
            wbd_t = pp.tile([128, NCHK * L * 128], F16)
            yo = pp.tile([128, GP], F16)
            nc.sync.dma_start(x_t[:], x8[:])
            nc.sync.dma_start(gidx_t[:], gidx[:])
            nc.sync.dma_start(yinv_t[:], yinv[:])
            for i in range(NCHK * L):
                nc.sync.dma_start(wbd_t[:, i * 128 : (i + 1) * 128], wbd[i])

            # Flattened (iteration, chunk) stream with the un-sort lagging
            # LAG chunk-slots behind its sort: the matmul->copy chain
            # (~3 us) finishes while Pool streams the next LAG scatters, so
            # the Pool engine never stalls waiting for conv output.
            LAG = 3
            pending = []  # (ys_tile, chunk_idx) awaiting un-sort

            def emit_ls3(ys_t, c):
                nc.gpsimd.local_scatter(
                    out_ap=yo[:, c * CHK : (c + 1) * CHK],
                    data_ap=ys_t[:],
                    idxs_ap=yinv_t[:, c * M : (c + 1) * M],
                    channels=128,
                    num_elems=CHK,
                    num_idxs=M,
                )

            for _ in range(iters):
                for c in range(NCHK):
                    # phase 1: scatter chunk pixels into rank-slot space
                    xs = xsp.tile([128, M], F16, tag="xs")
                    nc.gpsimd.local_scatter(
                        out_ap=xs[:],
                        data_ap=x_t[:, c * CHK : (c + 1) * CHK],
                        idxs_ap=gidx_t[:, c * CHK : (c + 1) * CHK],
                        channels=128,
                        num_elems=M,
                        num_idxs=CHK,
                    )
                    if len(pending) >= LAG:
                        emit_ls3(*pending.pop(0))
                    # phase 2: per-rank block-diag matmul
                    pm = pmp.tile([128, M], F32, tag="pm")
                    for r in range(L):
                        cap = profile[r]
                        if cap == 0:
                            continue
                        w_off = (c * L + r) * 128
                        nc.tensor.matmul(
                            out=pm[:, offs[r] : offs[r] + cap],
                            lhsT=wbd_t[:, w_off : w_off + 128],
                            rhs=xs[:, offs[r] : offs[r] + cap],
                            start=True,
                            stop=True,
                        )
                    ys = ysp.tile([128, M], F16, tag="ys")
                    if c % 2 == 0:
                        nc.vector.tensor_copy(out=ys[:], in_=pm[:])
                    else:
                        nc.scalar.copy(out=ys[:], in_=pm[:])
                    pending.append((ys, c))
            for ys_t, c in pending:
                emit_ls3(ys_t, c)
            nc.sync.dma_start(y8[:], yo[:])
    nc.compile()
    return nc


def _make_runner(nc):
    bass2jax.install_neuronx_cc_hook()
    partition_name = nc.partition_id_tensor.name if nc.partition_id_tensor else None
    in_names, out_names, out_avals, zero_outs = [], [], [], []
    for alloc in nc.m.functions[0].allocations:
        if not isinstance(alloc, mybir.MemoryLocationSet):
            continue
        name = alloc.memorylocations[0].name
        if alloc.kind == "ExternalInput":
            if name != partition_name:
                in_names.append(name)
        elif alloc.kind == "ExternalOutput":
            shape = tuple(alloc.tensor_shape)
            dtype = mybir.dt.np(alloc.dtype)
            out_names.append(name)
            out_avals.append(jax.core.ShapedArray(shape, dtype))
            zero_outs.append(np.zeros(shape, dtype))
    n_params = len(in_names)
    in_names_full = in_names + out_names + ([partition_name] if partition_name else [])

    def _body(*args):
        operands = list(args)
        if partition_name is not None:
            operands.append(bass2jax.partition_id_tensor())
        outs = bass2jax._bass_exec_p.bind(
            *operands,
            out_avals=tuple(out_avals),
            in_names=tuple(in_names_full),
            out_names=tuple(out_names),
            lowering_input_output_aliases=(),
            sim_require_finite=False,
            sim_require_nnan=False,
            nc=nc,
        )
        return tuple(outs)

    devices = jax.devices()[:N_CORES]
    mesh = Mesh(np.asarray(devices), ("core",))
    nin = n_params + len(out_names)
    sharded = jax.jit(
        shard_map(
            _body,
            mesh=mesh,
            in_specs=(PartitionSpec("core"),) * nin,
            out_specs=(PartitionSpec("core"),) * len(out_names),
            check_rep=False,
        ),
        keep_unused=True,
    )

    def run(in_maps):
        per_core = [[np.asarray(m[name]) for name in in_names] for m in in_maps]
        concat_in = [
            np.concatenate([per_core[c][i] for c in range(N_CORES)], axis=0)
            for i in range(n_params)
        ]
        concat_zeros = [
            np.zeros((N_CORES * z.shape[0], *z.shape[1:]), z.dtype) for z in zero_outs
        ]
        out_arrs = sharded(*concat_in, *concat_zeros)
        out_arrs = [np.asarray(a) for a in out_arrs]
        return [
            {
                name: out_arrs[i].reshape(N_CORES, *out_avals[i].shape)[c]
                for i, name in enumerate(out_names)
            }
            for c in range(N_CORES)
        ]

    return run


_CACHE = {}


def _get_runner(iters=1, profile=DEFAULT_PROFILE):
    key = (iters, profile)
    if key not in _CACHE:
        nc = _build_module(iters, profile)
        _CACHE[key] = _make_runner(nc)
    return _CACHE[key]


def _needed_profile(labels):
    """Elementwise-max of descending per-chunk label counts over all chunks."""
    prof = np.zeros(L, np.int64)
    for k in range(N_CORES):
        b, hh = k // 2, (k % 2) * 128
        lab = np.asarray(labels[b, hh : hh + 128, :]).reshape(NPIX)
        for g in range(NG):
            for c in range(NCHK):
                cnt = np.bincount(
                    lab[g * GP + c * CHK : g * GP + (c + 1) * CHK], minlength=L
                )
                prof = np.maximum(prof, np.sort(cnt)[::-1])
    return prof


def _prep_core_inputs(xc, lc, weight, profile=DEFAULT_PROFILE):
    """xc [C, 128, W] f32, lc [128, W] i32 -> per-core input dict."""
    offs, M = _layout(profile)
    x_flat = np.asarray(xc).reshape(C, NPIX)  # pixel = h_local*W + w
    x8 = np.ascontiguousarray(
        x_flat.reshape(C, NG, GP).transpose(1, 0, 2).reshape(128, GP)
    ).astype(np.float16)
    lab = np.asarray(lc).reshape(NPIX)
    w16 = np.asarray(weight).astype(np.float16)

    gidx = np.zeros((128, GP), np.int16)
    yinv = np.zeros((128, NCHK * M), np.int16)
    wbd = np.zeros((NCHK * L, 128, 128), np.float16)
    offs_a = np.asarray(offs, np.int64)
    for g in range(NG):
        for c in range(NCHK):
            lg = lab[g * GP + c * CHK : g * GP + (c + 1) * CHK]
            counts = np.bincount(lg, minlength=L)
            rank_order = np.argsort(-counts, kind="stable")  # rank -> label
            rank_of = np.empty(L, np.int64)
            rank_of[rank_order] = np.arange(L)
            if np.any(counts[rank_order] > np.asarray(profile)):
                raise RuntimeError("per-rank label count exceeds profile")
            order = np.argsort(lg, kind="stable")
            lab_sorted = lg[order]
            starts = np.concatenate([[0], np.cumsum(counts)[:-1]])
            rank_within = np.arange(CHK) - starts[lab_sorted]
            slot_sorted = offs_a[rank_of[lab_sorted]] + rank_within
            slot = np.empty(CHK, np.int16)
            slot[order] = slot_sorted.astype(np.int16)
            inv = np.full(M, -1, np.int16)
            inv[slot_sorted] = order.astype(np.int16)
            gidx[g * 16 : (g + 1) * 16, c * CHK : (c + 1) * CHK] = slot[None, :]
            yinv[g * 16 : (g + 1) * 16, c * M : (c + 1) * M] = inv[None, :]
            # block-diag stationary for this group's rank-r label
            for r in range(L):
                wt = w16[rank_order[r]].T  # lhsT[(g,ch),(g,o)] = W[l, o, ch]
                wbd[c * L + r, g * 16 : g * 16 + 16, g * 16 : g * 16 + 16] = wt
    return {
        "x8": x8,
        "gidx": gidx,
        "yinv": yinv,
        "wbd": wbd,
    }


def kernel(x, labels, weight, bias):
    x = np.asarray(x, dtype=np.float32)
    labels = np.asarray(labels, dtype=np.int32)
    weight = np.asarray(weight, dtype=np.float32)
    bias = np.asarray(bias, dtype=np.float32)

    needed = _needed_profile(labels)
    profile = DEFAULT_PROFILE
    if np.any(needed > np.asarray(DEFAULT_PROFILE)):
        # JIT-specialize for unusually skewed label distributions
        profile = tuple((np.maximum(needed, 2) + 1) // 2 * 2)

    run = _get_runner(1, profile)
    in_maps = []
    for k in range(N_CORES):
        b, hh = k // 2, (k % 2) * 128
        in_maps.append(
            _prep_core_inputs(
                x[b, :, hh : hh + 128, :], labels[b, hh : hh + 128, :], weight, profile
            )
        )
    res = run(in_maps)

    y = np.empty((B, C, H, W), dtype=np.float32)
    for k in range(N_CORES):
        b, hh = k // 2, (k % 2) * 128
        yk = (
            res[k]["y8"].astype(np.float32)
            .reshape(NG, C, GP)
            .transpose(1, 0, 2)
            .reshape(C, 128, W)
        )
        y[b, :, hh : hh + 128, :] = yk
    if np.any(bias):
        y += bias[labels][:, None, :, :]
    return y
